# revision 53
# baseline (speedup 1.0000x reference)
"""Trainium2 Bass kernel for an attention block with softmax over the QUERY axis.

Reference computation (per batch b):
    Q = x_b @ Wq^T + bq ; K = x_b @ Wk^T + bk ; V = x_b @ Wv^T + bv
    S = Q @ K^T / sqrt(256)
    attn = softmax(S, axis over queries)      # couples rows, not columns
    out_b = attn @ V

Algebraic facts exploited:
  * softmax over q removes any score term constant along q => bq drops out.
  * S^T = K Q^T = (x Wk^T + bk) Wq x^T = x P x^T + g x^T with
    P = Wk^T Wq and g = bk Wq precomputed ON HOST (weights only).  So the
    only projections on-core are GT = P^T x^T + g and V = x Wv^T + bv.

Precision strategy (validated in numpy sim against the fixed-seed inputs):
  * scores matmul bf16 (fp8 fails the 2e-2 gate: sim rel-err 0.021),
  * exp output and V/s quantized to fp8-e4m3 and the attn@V ("AV") matmul
    run in DoubleRow perf mode => ~1.8x faster than bf16 on that matmul,
  * V is pre-scaled by 2^15 (folded into Wv/bv on host) so V/s stays in
    fp8 normal range; output is fp16, host divides by 2^15.
    Sim: rel err 0.0108 (budget 2e-2).

Sharding over 8 NeuronCores: core m handles batch b = m // 2 and the
key/value half h = m % 2.  The host passes x_b^T with the query axis
rotated so the core's 2048 keys are always columns 0:2048; each core holds
the full query range for its batch, so the softmax over queries is fully
local.  Each core produces a partial output out^T (sum over its 2048 keys);
the host rotates back and adds the two partials per batch. No collectives.

On-core dataflow:
    xT   (128, 2, 4096) bf16   host-transposed input, d on (partition, plane)
    GT   (2 x [128, 2048]) bf16 = P^T x^T + g      (d' on partitions)
    V    (16 x [128, 256]) bf16 = x Wv^T + bv      (k on partitions), x2^15
    ST   (k, q) tiles = GT^T slices @ xT           (scores, PSUM, bf16 mm)
    e8   (8 x [128, 2, 4096]) fp8e4 = exp(ST/16)   (ACT, no max-subtraction:
         |S/16| < ~2.2); row sums via the ACT accumulator
    V8   (8 x [128, 2, 256]) fp8e4 = V / s_k       (per-partition scale)
    out^T (128, 2, 4096) fp16: DoubleRow matmuls V8-pair^T @ e8-pair,
         k-split 4+4 pairs: the first half interleaved under the second
         half's scores phase, partials added on DVE.

Scheduling notes (trace-driven):
  * DMA descriptor rate (~170GB/s on the sync HW-DGE queue; the ACT-queue
    DGE is ~5x slower) makes the xT load the startup bottleneck: chunks go
    in need order with the longest contiguous runs, and k-tile 0's first
    query half is scored from just the first 512KB chunk.
  * Phase 1a runs all k-tiles' first query halves before any second half,
    so the exp chain never waits on the late xT chunks.
  * Junk warm-up matmuls bridge the DMA wait so the PE HAM clock gate
    stays at 2.4 GHz when real matmuls start.
  * P1b score slots are 3x[128,1024] so the PE never stalls on ACT
    draining a slot; AV partials keep 2 more psum banks.
  * Measured on 8 axon trn2 cores: ~125 us HW exec, rel err ~1.06%
    (chip-state run variance ~ +-3 us).
"""

import numpy as np
import ml_dtypes

import concourse.bass as bass
import concourse.tile as tile
from concourse import bacc, mybir
from concourse.bass_utils import run_bass_kernel_spmd

BF16 = ml_dtypes.bfloat16
F16 = mybir.dt.float16
F32 = mybir.dt.float32
BF = mybir.dt.bfloat16
F8 = mybir.dt.float8e4

B, S, D = 4, 4096, 256
NCORES = 8
KH = S // 2          # 2048 keys per core
NKT = KH // 128      # 16 key tiles
NPAIR = NKT // 2     # 8 key pairs (DoubleRow processes 2 k-tiles per mm)
NQG = S // 512       # 8 query groups of 512
VSCALE = 2.0 ** 15   # folded into Wv/bv on host; host divides out at the end

EXP = mybir.ActivationFunctionType.Exp
IDENT = mybir.ActivationFunctionType.Identity
AX = mybir.AxisListType.X
ADD = mybir.AluOpType.add
DR = mybir.MatmulPerfMode.DoubleRow


def _emit(tc, xT, wpT, bvr, out):
    nc = tc.nc

    with tc.tile_pool(name="const", bufs=1) as cpool, \
         tc.tile_pool(name="big", bufs=1) as bpool, \
         tc.tile_pool(name="work", bufs=4) as wpool:

        # ---- input loads: w3 on sync queue first; the xt chunks the first
        # scores tile needs go on the (otherwise idle) gpsimd DMA queue so
        # they transfer in parallel with w3/bv.  wpT column 512 carries the
        # GT bias g (bf16 is plenty: |g| ~ 0.02, scores ~ 10).
        w3 = cpool.tile([128, 2 * (2 * D + 1)], BF, name="w3", tag="w3")
        bv_sb = cpool.tile([1, D], BF, name="bv", tag="bv")
        ones = cpool.tile([1, 128], BF, name="ones", tag="ones")
        nc.vector.memset(ones, 1.0)
        junk = cpool.tile([128, 512], BF, name="junk", tag="junk")
        nc.vector.memset(junk, 1.0)

        # xT as one (128, 2, 4096) tile: [:, t, :] = d-rows [128t, 128(t+1))
        # DMA descriptor rate is the startup bottleneck (~60ns/descriptor),
        # so chunks use the longest contiguous DRAM runs available (4KB =
        # 2048 bf16 cols) split plane-wise across the two HW-DGE queues.
        # The key columns + first query half (cols 0:2048) land first; the
        # second query half is only needed by the (reordered) B-half scores.
        xt3 = cpool.tile([128, 2, S], BF, name="xt3", tag="xt3")
        xt_in = xT.rearrange("(t p) s -> p t s", p=128)
        # critical path: P-weights (w3 cols 0:514), then xt cols 0:2048 in
        # two both-plane chunks (first scores tile starts after the first).
        # The slow ACT-queue DGE (~5x slower) carries only the last-needed
        # chunk; everything else rides the fast sync queue in need order.
        nc.sync.dma_start(w3[:, 0:514], wpT[:, 0:514])
        nc.sync.dma_start(xt3[:, :, 0:1024], xt_in[:, :, 0:1024])
        nc.sync.dma_start(xt3[:, :, 1024:2048], xt_in[:, :, 1024:2048])
        nc.sync.dma_start(w3[:, 514:1026], wpT[:, 514:1026])
        nc.sync.dma_start(bv_sb, bvr)
        nc.sync.dma_start(xt3[:, :, 2048:3072], xt_in[:, :, 2048:3072])
        nc.sync.dma_start(xt3[:, :, 3072:4096], xt_in[:, :, 3072:4096])

        def wp(t, h):                    # P rows [128t,..), d' cols [128h,..)
            return w3[:, 256 * t + 128 * h:256 * t + 128 * (h + 1)]

        def wv(t):                       # Wv^T rows [128t,..) (x VSCALE)
            return w3[:, 514 + 256 * t:514 + 256 * (t + 1)]

        g_sb = cpool.tile([128, 2, 1], F32, name="gsb", tag="gsb")
        nc.vector.tensor_copy(g_sb[:, 0, 0:1], w3[:, 512:513])
        nc.vector.tensor_copy(g_sb[:, 1, 0:1], w3[:, 513:514])

        def gb(h):                       # GT bias g for d'-half h
            return g_sb[:, h, 0:1]

        # ---- persistent tiles ----
        GT_sb = [bpool.tile([128, KH], BF, name=f"GT{h}", tag=f"GT{h}")
                 for h in range(2)]
        Vb_sb = [bpool.tile([128, D], BF, name=f"Vb{k}", tag=f"Vb{k}")
                 for k in range(NKT)]
        e8_sb = [bpool.tile([128, 2, S], F8, name=f"e8_{t}", tag=f"e8_{t}")
                 for t in range(NPAIR)]
        V8_sb = [bpool.tile([128, 2, D], F8, name=f"V8_{t}", tag=f"V8_{t}")
                 for t in range(NPAIR)]
        part_sb = [bpool.tile([128, 512], F16, name=f"pt{j}", tag=f"pt{j}")
                   for j in range(2 * NQG)]
        outT_sb = bpool.tile([128, 2, S], F16, name="outT", tag="outT")
        bvb_sb = cpool.tile([128, D], BF, name="bvb", tag="bvb")
        # per-tile exp partial sums, persistent (P1a A/B halves are split in
        # time so wpool rotation would clobber them); one tile per k-tile so
        # the dependency tracker never serializes across tiles
        sp_sb = [cpool.tile([128, 4], F32, name=f"sp{k}", tag=f"sp{k}")
                 for k in range(NKT)]

        # ---- emitters ----
        def gt_chunk(sub, h, cs):        # GT[h][:, cs] from x columns cs
            nc.tensor.matmul(sub[:, 0:cs.stop - cs.start], wp(0, h),
                             xt3[:, 0, cs], start=True, stop=False)
            nc.tensor.matmul(sub[:, 0:cs.stop - cs.start], wp(1, h),
                             xt3[:, 1, cs], start=False, stop=True)
            nc.vector.tensor_scalar_add(GT_sb[h][:, cs],
                                        sub[:, 0:cs.stop - cs.start], gb(h))

        def v_group(sub, k):             # V rows [128k,..) + bias (x VSCALE)
            ks = slice(128 * k, 128 * (k + 1))
            nc.tensor.matmul(sub, xt3[:, 0, ks], wv(0), start=True, stop=False)
            nc.tensor.matmul(sub, xt3[:, 1, ks], wv(1), start=False, stop=True)
            nc.vector.tensor_tensor(Vb_sb[k], sub, bvb_sb, op=ADD)

        def score_mms(slot, kt, q0, width):
            for g2 in range(width // 512):
                sub = slot[:, 512 * g2:512 * (g2 + 1)]
                qs = slice(q0 + 512 * g2, q0 + 512 * (g2 + 1))
                nc.tensor.matmul(sub, GT_sb[0][:, 128 * kt:128 * (kt + 1)],
                                 xt3[:, 0, qs], start=True, stop=False)
                nc.tensor.matmul(sub, GT_sb[1][:, 128 * kt:128 * (kt + 1)],
                                 xt3[:, 1, qs], start=False, stop=True)

        def emit_exp(slot, kt, q0, width, sidx):
            t, pl = divmod(kt, 2)
            nc.scalar.activation(e8_sb[t][:, pl, q0:q0 + width],
                                 slot[:, 0:width], EXP, scale=1.0 / 16.0,
                                 accum_out=sp_sb[kt][:, sidx:sidx + 1])

        def emit_row_scale(kt, nparts):
            t, pl = divmod(kt, 2)
            ssum = wpool.tile([128, 1], F32, name="ssum", tag="ssum")
            nc.vector.reduce_sum(ssum, sp_sb[kt][:, 0:nparts], axis=AX)
            rs = wpool.tile([128, 1], F32, name="rs", tag="rs")
            nc.vector.reciprocal(rs, ssum)
            nc.vector.tensor_scalar_mul(V8_sb[t][:, pl, :], Vb_sb[kt], rs)

        def emit_av(pool, qg, h, pairs, accumulate_part):
            # out^T[d-half h, 512 queries] partial over the given key pairs
            pa = pool.tile([128, 512], F32, name="pav", tag="pav")
            qs = slice(512 * qg, 512 * (qg + 1))
            for n, t in enumerate(pairs):
                nc.tensor.matmul(pa, V8_sb[t][:, :, 128 * h:128 * (h + 1)],
                                 e8_sb[t][:, :, qs], start=(n == 0),
                                 stop=(n == len(pairs) - 1), perf_mode=DR)
            if not accumulate_part:
                nc.vector.tensor_copy(part_sb[2 * qg + h], pa)
            else:
                nc.vector.tensor_tensor(outT_sb[:, h, qs], pa,
                                        part_sb[2 * qg + h], op=ADD)

        # ====== phase 0: projections (+ first score tile interleaved) ======
        psa_cm = tc.tile_pool(name="psa", bufs=1, space="PSUM")
        psa = psa_cm.__enter__()

        with tc.tile_pool(name="ps0", bufs=4, space="PSUM") as ps0:
            def slot0():
                return ps0.tile([128, 512], F32, name="ps0t", tag="ps0t")

            # junk matmuls only bridge the gap until the first xT chunk
            # lands; real MMs then keep the PE HAM activity window fed
            warm = slot0()
            for _ in range(9):
                nc.tensor.matmul(warm, junk[:, 0:128], junk,
                                 start=True, stop=True)

            # critical path to the first exp: GT cols 0:128 (mini chunk),
            # then k-tile 0's first query half in two 1024-wide pieces so
            # scoring starts after only 512KB of xT has landed
            for h in range(2):
                gt_chunk(slot0(), h, slice(0, 128))
            sA = psa.tile([128, 2048], F32, name="psat", tag="psat")
            score_mms(sA, 0, 0, 1024)
            emit_exp(sA, 0, 0, 1024, 0)
            for h in range(2):
                gt_chunk(slot0(), h, slice(128, 512))
            score_mms(sA[:, 1024:2048], 0, 1024, 1024)
            emit_exp(sA[:, 1024:2048], 0, 1024, 1024, 1)

            for h in range(2):
                gt_chunk(slot0(), h, slice(512, 1024))
            pt = slot0()
            nc.tensor.matmul(pt[:, 0:D], ones, bv_sb, start=True, stop=True)
            nc.vector.tensor_copy(bvb_sb, pt[:, 0:D])
            for k in range(8):
                v_group(slot0()[:, 0:D], k)
            for kb in range(2, 4):
                for h in range(2):
                    gt_chunk(slot0(), h, slice(512 * kb, 512 * (kb + 1)))
            for k in range(8, NKT):
                v_group(slot0()[:, 0:D], k)

        # == phase 1a: scores+exp k 0..6, all A-halves (q 0:2048) first so
        # the exp chain never waits on the second xT query chunk ===========
        psa2_cm = tc.tile_pool(name="psa2", bufs=1, space="PSUM")
        psa2 = psa2_cm.__enter__()
        slot_ctr = [1]

        def slot_next():
            pool = psa if slot_ctr[0] % 2 == 0 else psa2
            slot_ctr[0] += 1
            return pool.tile([128, 2048], F32, name="psat", tag="psat")

        for kt in range(1, 7):
            s = slot_next()
            score_mms(s, kt, 0, 2048)
            emit_exp(s, kt, 0, 2048, 0)
        for kt in range(0, 7):
            s = slot_next()
            score_mms(s, kt, 2048, 2048)
            emit_exp(s, kt, 2048, 2048, 2 if kt == 0 else 1)
            emit_row_scale(kt, 3 if kt == 0 else 2)
        psa2_cm.__exit__(None, None, None)
        psa_cm.__exit__(None, None, None)

        # == phase 1b: scores+exp k 7..15 + AV over key pairs 0..3 ==========
        # (tile 7 rides here because this phase has ACT slack; the ACT-paced
        # phase 1a stays one tile shorter)
        psav_cm = tc.tile_pool(name="psav", bufs=2, space="PSUM")
        psav = psav_cm.__enter__()
        with tc.tile_pool(name="psb", bufs=3, space="PSUM") as psb:
            for kt in range(7, NKT):
                qg = kt - 8
                for quarter in range(4):
                    s = psb.tile([128, 1024], F32, name="psbt", tag="psbt")
                    score_mms(s, kt, 1024 * quarter, 1024)
                    emit_exp(s, kt, 1024 * quarter, 1024, quarter)
                    # AV interleave: keeps PE fed while ACT drains the slots
                    if kt >= 8:
                        if quarter == 1:
                            emit_av(psav, qg, 0, range(4),
                                    accumulate_part=False)
                        elif quarter == 3:
                            emit_av(psav, qg, 1, range(4),
                                    accumulate_part=False)
                emit_row_scale(kt, 4)

        # ========= phase 2: AV over key pairs 4..7 + partial add ===========
        # h-major so output DMAs use 4KB-contiguous DRAM runs per qg-pair.
        # The first two groups run from the still-open psav pool so their
        # pair-4..6 matmuls can start under the tail of the last exp instead
        # of waiting for the score-slot banks to drain.
        out3 = out.rearrange("(t p) q -> p t q", p=128)
        with tc.tile_pool(name="psav2", bufs=6, space="PSUM") as psav2:
            for h in range(2):
                for qg in range(NQG):
                    pool = psav if (h == 0 and qg < 2) else psav2
                    emit_av(pool, qg, h, range(4, NPAIR),
                            accumulate_part=True)
                    if qg % 2 == 1:
                        qs = slice(512 * (qg - 1), 512 * (qg + 1))
                        nc.sync.dma_start(out3[:, h, qs], outT_sb[:, h, qs])
        psav_cm.__exit__(None, None, None)


def build():
    nc = bacc.Bacc("TRN2", target_bir_lowering=False, debug=False)
    xT = nc.dram_tensor("xT", [D, S], BF, kind="ExternalInput").ap()
    wpT = nc.dram_tensor("wpT", [128, 2 * (2 * D + 1)], BF,
                         kind="ExternalInput").ap()
    bvr = nc.dram_tensor("bvr", [1, D], BF, kind="ExternalInput").ap()
    out = nc.dram_tensor("out", [D, S], F16, kind="ExternalOutput").ap()

    with tile.TileContext(nc) as tc:
        _emit(tc, xT, wpT, bvr, out)
    nc.compile()
    return nc


_NC = None


def _get_nc():
    global _NC
    if _NC is None:
        _NC = build()
    return _NC


def make_in_maps(x, Wq, bq, Wk, bk, Wv, bv):
    # bq cancels under the softmax-over-queries (see module docstring)
    Wq64 = np.asarray(Wq, np.float64)
    Wk64 = np.asarray(Wk, np.float64)
    P = (Wk64.T @ Wq64).astype(np.float32)          # [d2, d]
    g = (np.asarray(bk, np.float64) @ Wq64).astype(np.float32).reshape(D, 1)
    # pack to the on-core SBUF layout [128, 1026]:
    #   [P_t0 | P_t1 | g_t0 | g_t1 | wv_t0 | wv_t1]  (t = d-row half)
    # so the weight DMA is contiguous runs and the P/g part loads first
    WvT = np.asarray(Wv, np.float32).T * VSCALE
    wpT = np.empty((128, 1026), np.float32)
    wpT[:, 0:256] = P[0:128]
    wpT[:, 256:512] = P[128:256]
    wpT[:, 512] = g[0:128, 0]
    wpT[:, 513] = g[128:256, 0]
    wpT[:, 514:770] = WvT[0:128]
    wpT[:, 770:1026] = WvT[128:256]
    wpT = np.ascontiguousarray(wpT).astype(BF16)
    bvr = (np.asarray(bv, np.float32) * VSCALE).reshape(1, D).astype(BF16)
    in_maps = []
    for core in range(NCORES):
        b, h = divmod(core, 2)
        xTb = np.asarray(x[b]).T.astype(BF16)
        if h:  # rotate so this core's keys are always columns 0:KH
            xTb = np.concatenate([xTb[:, KH:], xTb[:, :KH]], axis=1)
        in_maps.append({
            "xT": np.ascontiguousarray(xTb),
            "wpT": wpT, "bvr": bvr,
        })
    return in_maps


def run(x, Wq, bq, Wk, bk, Wv, bv, trace=False):
    """Run on the 8 cores; returns (full_output, BassKernelResults)."""
    nc = _get_nc()
    in_maps = make_in_maps(x, Wq, bq, Wk, bk, Wv, bv)
    res = run_bass_kernel_spmd(nc, in_maps, core_ids=list(range(NCORES)),
                               trace=trace)
    outs = []
    for b in range(B):
        p0 = res.results[2 * b]["out"].astype(np.float32)      # [D, S]
        p1 = res.results[2 * b + 1]["out"].astype(np.float32)  # [D, S] rotated
        p1 = np.concatenate([p1[:, KH:], p1[:, :KH]], axis=1)  # undo rotation
        outs.append((p0 + p1).T * (1.0 / VSCALE))
    return np.stack(outs).astype(np.float32), res


def kernel(x, Wq, bq, Wk, bk, Wv, bv):
    full, _ = run(x, Wq, bq, Wk, bk, Wv, bv, trace=False)
    return full


# revision 56
# speedup vs baseline: 1.2038x; 1.2038x over previous
"""Trainium2 Bass kernel for an attention block with softmax over the QUERY axis.

Reference computation (per batch b):
    Q = x_b @ Wq^T + bq ; K = x_b @ Wk^T + bk ; V = x_b @ Wv^T + bv
    S = Q @ K^T / sqrt(256)
    attn = softmax(S, axis over queries)      # couples rows, not columns
    out_b = attn @ V

Algebraic facts exploited:
  * softmax over q removes any score term constant along q => bq drops out.
  * S^T = K Q^T = (x Wk^T + bk) Wq x^T = x P x^T + g x^T with
    P = Wk^T Wq and g = bk Wq precomputed ON HOST (weights only).  So the
    only projections on-core are GT = P^T x^T + g and V = x Wv^T + bv.

Precision strategy (validated in numpy sim against the fixed-seed inputs):
  * scores matmul bf16 (fp8 fails the 2e-2 gate: sim rel-err 0.021),
  * exp output and V/s quantized to fp8-e4m3 and the attn@V ("AV") matmul
    run in DoubleRow perf mode => ~1.8x faster than bf16 on that matmul,
  * V is pre-scaled by 2^15 (folded into Wv/bv on host) so V/s stays in
    fp8 normal range; output is fp16, host divides by 2^15.
    Sim: rel err 0.0108 (budget 2e-2).

Sharding over 8 NeuronCores: core m handles batch b = m // 2 and the
key/value half h = m % 2.  The host passes x_b^T with the query axis
rotated so the core's 2048 keys are always columns 0:2048; each core holds
the full query range for its batch, so the softmax over queries is fully
local.  Each core produces a partial output out^T (sum over its 2048 keys);
the host rotates back and adds the two partials per batch. No collectives.

On-core dataflow:
    xT   (128, 2, 4096) bf16   host-transposed input, d on (partition, plane)
    GT   (2 x [128, 2048]) bf16 = P^T x^T + g      (d' on partitions)
    V    (16 x [128, 256]) bf16 = x Wv^T + bv      (k on partitions), x2^15
    ST   (k, q) tiles = GT^T slices @ xT           (scores, PSUM, bf16 mm)
    e8   (8 x [128, 2, 4096]) fp8e4 = exp(ST/16)   (ACT, no max-subtraction:
         |S/16| < ~2.2); row sums via the ACT accumulator
    V8   (8 x [128, 2, 256]) fp8e4 = V / s_k       (per-partition scale)
    out^T (128, 2, 4096) fp16: DoubleRow matmuls V8-pair^T @ e8-pair,
         k-split 4+4 pairs: the first half interleaved under the second
         half's scores phase, partials added on DVE.

Scheduling notes (trace-driven):
  * DMA descriptor rate (~170GB/s on the sync HW-DGE queue; the ACT-queue
    DGE is ~5x slower) makes the xT load the startup bottleneck: chunks go
    in need order with the longest contiguous runs, and k-tile 0's first
    query half is scored from just the first 512KB chunk.
  * Phase 1a runs all k-tiles' first query halves before any second half,
    so the exp chain never waits on the late xT chunks.
  * Junk warm-up matmuls bridge the DMA wait so the PE HAM clock gate
    stays at 2.4 GHz when real matmuls start.
  * P1b score slots are 3x[128,1024] so the PE never stalls on ACT
    draining a slot; AV partials keep 2 more psum banks.
  * Measured on 8 axon trn2 cores: ~125 us HW exec, rel err ~1.06%
    (chip-state run variance ~ +-3 us).
"""

import numpy as np
import ml_dtypes

import concourse.bass as bass
import concourse.tile as tile
from concourse import bacc, mybir
from concourse.bass_utils import run_bass_kernel_spmd

BF16 = ml_dtypes.bfloat16
F16 = mybir.dt.float16
F32 = mybir.dt.float32
BF = mybir.dt.bfloat16
F8 = mybir.dt.float8e4

B, S, D = 4, 4096, 256
NCORES = 8
KH = S // 2          # 2048 keys per core
NKT = KH // 128      # 16 key tiles
NPAIR = NKT // 2     # 8 key pairs (DoubleRow processes 2 k-tiles per mm)
NQG = S // 512       # 8 query groups of 512
VSCALE = 2.0 ** 15   # folded into Wv/bv on host; host divides out at the end

EXP = mybir.ActivationFunctionType.Exp
IDENT = mybir.ActivationFunctionType.Identity
AX = mybir.AxisListType.X
ADD = mybir.AluOpType.add
DR = mybir.MatmulPerfMode.DoubleRow


def _emit(tc, xT, wpT, bvr, out):
    nc = tc.nc

    with tc.tile_pool(name="const", bufs=1) as cpool, \
         tc.tile_pool(name="big", bufs=1) as bpool, \
         tc.tile_pool(name="work", bufs=4) as wpool:

        # ---- input loads: w3 on sync queue first; the xt chunks the first
        # scores tile needs go on the (otherwise idle) gpsimd DMA queue so
        # they transfer in parallel with w3/bv.  wpT column 512 carries the
        # GT bias g (bf16 is plenty: |g| ~ 0.02, scores ~ 10).
        w3 = cpool.tile([128, 2 * (2 * D + 1)], BF, name="w3", tag="w3")
        bv_sb = cpool.tile([1, D], BF, name="bv", tag="bv")
        ones = cpool.tile([1, 128], BF, name="ones", tag="ones")
        nc.vector.memset(ones, 1.0)
        junk = cpool.tile([128, 512], BF, name="junk", tag="junk")
        nc.vector.memset(junk, 1.0)

        # xT as one (128, 2, 4096) tile: [:, t, :] = d-rows [128t, 128(t+1))
        # DMA descriptor rate is the startup bottleneck (~60ns/descriptor),
        # so chunks use the longest contiguous DRAM runs available (4KB =
        # 2048 bf16 cols) split plane-wise across the two HW-DGE queues.
        # The key columns + first query half (cols 0:2048) land first; the
        # second query half is only needed by the (reordered) B-half scores.
        xt3 = cpool.tile([128, 2, S], BF, name="xt3", tag="xt3")
        xt_in = xT.rearrange("(t p) s -> p t s", p=128)
        # critical path: P-weights (w3 cols 0:514), then xt cols 0:2048 in
        # two both-plane chunks (first scores tile starts after the first).
        # The slow ACT-queue DGE (~5x slower) carries only the last-needed
        # chunk; everything else rides the fast sync queue in need order.
        nc.sync.dma_start(w3[:, 0:514], wpT[:, 0:514])
        nc.sync.dma_start(xt3[:, :, 0:1024], xt_in[:, :, 0:1024])
        nc.sync.dma_start(xt3[:, :, 1024:2048], xt_in[:, :, 1024:2048])
        nc.sync.dma_start(w3[:, 514:1026], wpT[:, 514:1026])
        nc.sync.dma_start(bv_sb, bvr)
        nc.sync.dma_start(xt3[:, :, 2048:3072], xt_in[:, :, 2048:3072])
        nc.sync.dma_start(xt3[:, :, 3072:4096], xt_in[:, :, 3072:4096])

        def wp(t, h):                    # P rows [128t,..), d' cols [128h,..)
            return w3[:, 256 * t + 128 * h:256 * t + 128 * (h + 1)]

        def wv(t):                       # Wv^T rows [128t,..) (x VSCALE)
            return w3[:, 514 + 256 * t:514 + 256 * (t + 1)]

        g_sb = cpool.tile([128, 2, 1], F32, name="gsb", tag="gsb")
        nc.vector.tensor_copy(g_sb[:, 0, 0:1], w3[:, 512:513])
        nc.vector.tensor_copy(g_sb[:, 1, 0:1], w3[:, 513:514])

        def gb(h):                       # GT bias g for d'-half h
            return g_sb[:, h, 0:1]

        # ---- persistent tiles ----
        GT_sb = [bpool.tile([128, KH], BF, name=f"GT{h}", tag=f"GT{h}")
                 for h in range(2)]
        Vb_sb = [bpool.tile([128, D], BF, name=f"Vb{k}", tag=f"Vb{k}")
                 for k in range(NKT)]
        e8_sb = [bpool.tile([128, 2, S], F8, name=f"e8_{t}", tag=f"e8_{t}")
                 for t in range(NPAIR)]
        V8_sb = [bpool.tile([128, 2, D], F8, name=f"V8_{t}", tag=f"V8_{t}")
                 for t in range(NPAIR)]
        part_sb = [bpool.tile([128, 512], F16, name=f"pt{j}", tag=f"pt{j}")
                   for j in range(2 * NQG)]
        outT_sb = bpool.tile([128, 2, S], F16, name="outT", tag="outT")
        bvb_sb = cpool.tile([128, D], BF, name="bvb", tag="bvb")
        # per-tile exp partial sums, persistent (P1a A/B halves are split in
        # time so wpool rotation would clobber them); one tile per k-tile so
        # the dependency tracker never serializes across tiles
        sp_sb = [cpool.tile([128, 4], F32, name=f"sp{k}", tag=f"sp{k}")
                 for k in range(NKT)]

        # ---- emitters ----
        def gt_chunk(sub, h, cs):        # GT[h][:, cs] from x columns cs
            nc.tensor.matmul(sub[:, 0:cs.stop - cs.start], wp(0, h),
                             xt3[:, 0, cs], start=True, stop=False)
            nc.tensor.matmul(sub[:, 0:cs.stop - cs.start], wp(1, h),
                             xt3[:, 1, cs], start=False, stop=True)
            nc.vector.tensor_scalar_add(GT_sb[h][:, cs],
                                        sub[:, 0:cs.stop - cs.start], gb(h))

        def v_group(sub, k):             # V rows [128k,..) + bias (x VSCALE)
            ks = slice(128 * k, 128 * (k + 1))
            nc.tensor.matmul(sub, xt3[:, 0, ks], wv(0), start=True, stop=False)
            nc.tensor.matmul(sub, xt3[:, 1, ks], wv(1), start=False, stop=True)
            nc.vector.tensor_tensor(Vb_sb[k], sub, bvb_sb, op=ADD)

        def score_mms(slot, kt, q0, width):
            for g2 in range(width // 512):
                sub = slot[:, 512 * g2:512 * (g2 + 1)]
                qs = slice(q0 + 512 * g2, q0 + 512 * (g2 + 1))
                nc.tensor.matmul(sub, GT_sb[0][:, 128 * kt:128 * (kt + 1)],
                                 xt3[:, 0, qs], start=True, stop=False)
                nc.tensor.matmul(sub, GT_sb[1][:, 128 * kt:128 * (kt + 1)],
                                 xt3[:, 1, qs], start=False, stop=True)

        def emit_exp(slot, kt, q0, width, sidx):
            t, pl = divmod(kt, 2)
            nc.scalar.activation(e8_sb[t][:, pl, q0:q0 + width],
                                 slot[:, 0:width], EXP, scale=1.0 / 16.0,
                                 accum_out=sp_sb[kt][:, sidx:sidx + 1])

        def emit_row_scale(kt, nparts):
            t, pl = divmod(kt, 2)
            ssum = wpool.tile([128, 1], F32, name="ssum", tag="ssum")
            nc.vector.reduce_sum(ssum, sp_sb[kt][:, 0:nparts], axis=AX)
            rs = wpool.tile([128, 1], F32, name="rs", tag="rs")
            nc.vector.reciprocal(rs, ssum)
            nc.vector.tensor_scalar_mul(V8_sb[t][:, pl, :], Vb_sb[kt], rs)

        def emit_av(pool, qg, h, pairs, accumulate_part):
            # out^T[d-half h, 512 queries] partial over the given key pairs
            pa = pool.tile([128, 512], F32, name="pav", tag="pav")
            qs = slice(512 * qg, 512 * (qg + 1))
            for n, t in enumerate(pairs):
                nc.tensor.matmul(pa, V8_sb[t][:, :, 128 * h:128 * (h + 1)],
                                 e8_sb[t][:, :, qs], start=(n == 0),
                                 stop=(n == len(pairs) - 1), perf_mode=DR)
            if not accumulate_part:
                nc.vector.tensor_copy(part_sb[2 * qg + h], pa)
            else:
                nc.vector.tensor_tensor(outT_sb[:, h, qs], pa,
                                        part_sb[2 * qg + h], op=ADD)

        # ====== phase 0: projections (+ first score tile interleaved) ======
        psa_cm = tc.tile_pool(name="psa", bufs=1, space="PSUM")
        psa = psa_cm.__enter__()

        with tc.tile_pool(name="ps0", bufs=4, space="PSUM") as ps0:
            def slot0():
                return ps0.tile([128, 512], F32, name="ps0t", tag="ps0t")

            # junk matmuls only bridge the gap until the first xT chunk
            # lands; real MMs then keep the PE HAM activity window fed
            warm = slot0()
            for _ in range(7):
                nc.tensor.matmul(warm, junk[:, 0:128], junk,
                                 start=True, stop=True)

            # critical path to the first exp: GT cols 0:128 (mini chunk),
            # then k-tile 0's first query half in two 1024-wide pieces so
            # scoring starts after only 512KB of xT has landed
            for h in range(2):
                gt_chunk(slot0(), h, slice(0, 128))
            sA = psa.tile([128, 2048], F32, name="psat", tag="psat")
            score_mms(sA, 0, 0, 1024)
            emit_exp(sA, 0, 0, 1024, 0)
            for h in range(2):
                gt_chunk(slot0(), h, slice(128, 512))
            score_mms(sA[:, 1024:2048], 0, 1024, 1024)
            emit_exp(sA[:, 1024:2048], 0, 1024, 1024, 1)

            for h in range(2):
                gt_chunk(slot0(), h, slice(512, 1024))
            pt = slot0()
            nc.tensor.matmul(pt[:, 0:D], ones, bv_sb, start=True, stop=True)
            nc.vector.tensor_copy(bvb_sb, pt[:, 0:D])
            for k in range(8):
                v_group(slot0()[:, 0:D], k)
            # k-tile 1's first half rides here (psa's buffer is free once
            # exp k0A2 drains): the exp chain starts ~5us earlier and the
            # phase-1a slot rotation stops bunching against ACT
            s1A = psa.tile([128, 2048], F32, name="psat", tag="psat")
            score_mms(s1A, 1, 0, 2048)
            emit_exp(s1A, 1, 0, 2048, 0)
            for kb in range(2, 4):
                for h in range(2):
                    gt_chunk(slot0(), h, slice(512 * kb, 512 * (kb + 1)))
            for k in range(8, NKT):
                v_group(slot0()[:, 0:D], k)

        # == phase 1a: scores+exp k 0..6, all A-halves (q 0:2048) first so
        # the exp chain never waits on the second xT query chunk ===========
        psa2_cm = tc.tile_pool(name="psa2", bufs=1, space="PSUM")
        psa2 = psa2_cm.__enter__()
        slot_ctr = [1]

        def slot_next():
            pool = psa if slot_ctr[0] % 2 == 0 else psa2
            slot_ctr[0] += 1
            return pool.tile([128, 2048], F32, name="psat", tag="psat")

        for kt in range(2, 7):
            s = slot_next()
            score_mms(s, kt, 0, 2048)
            emit_exp(s, kt, 0, 2048, 0)
        for kt in range(0, 7):
            s = slot_next()
            score_mms(s, kt, 2048, 2048)
            emit_exp(s, kt, 2048, 2048, 2 if kt == 0 else 1)
            emit_row_scale(kt, 3 if kt == 0 else 2)
        psa2_cm.__exit__(None, None, None)
        psa_cm.__exit__(None, None, None)

        # == phase 1b: scores+exp k 7..15 + AV over key pairs 0..3 ==========
        # (tile 7 rides here because this phase has ACT slack; the ACT-paced
        # phase 1a stays one tile shorter)
        psav_cm = tc.tile_pool(name="psav", bufs=2, space="PSUM")
        psav = psav_cm.__enter__()
        with tc.tile_pool(name="psb", bufs=3, space="PSUM") as psb:
            for kt in range(7, NKT):
                qg = kt - 8
                for quarter in range(4):
                    s = psb.tile([128, 1024], F32, name="psbt", tag="psbt")
                    score_mms(s, kt, 1024 * quarter, 1024)
                    emit_exp(s, kt, 1024 * quarter, 1024, quarter)
                    # AV interleave: keeps PE fed while ACT drains the slots
                    if kt >= 8:
                        if quarter == 1:
                            emit_av(psav, qg, 0, range(4),
                                    accumulate_part=False)
                        elif quarter == 3:
                            emit_av(psav, qg, 1, range(4),
                                    accumulate_part=False)
                emit_row_scale(kt, 4)

        # ========= phase 2: AV over key pairs 4..7 + partial add ===========
        # h-major so output DMAs use 4KB-contiguous DRAM runs per qg-pair.
        # The first two groups run from the still-open psav pool so their
        # pair-4..6 matmuls can start under the tail of the last exp instead
        # of waiting for the score-slot banks to drain.
        out3 = out.rearrange("(t p) q -> p t q", p=128)
        with tc.tile_pool(name="psav2", bufs=6, space="PSUM") as psav2:
            for h in range(2):
                for qg in range(NQG):
                    pool = psav if (h == 0 and qg < 2) else psav2
                    emit_av(pool, qg, h, range(4, NPAIR),
                            accumulate_part=True)
                    if qg % 2 == 1:
                        qs = slice(512 * (qg - 1), 512 * (qg + 1))
                        nc.sync.dma_start(out3[:, h, qs], outT_sb[:, h, qs])
        psav_cm.__exit__(None, None, None)


def build():
    nc = bacc.Bacc("TRN2", target_bir_lowering=False, debug=False)
    xT = nc.dram_tensor("xT", [D, S], BF, kind="ExternalInput").ap()
    wpT = nc.dram_tensor("wpT", [128, 2 * (2 * D + 1)], BF,
                         kind="ExternalInput").ap()
    bvr = nc.dram_tensor("bvr", [1, D], BF, kind="ExternalInput").ap()
    out = nc.dram_tensor("out", [D, S], F16, kind="ExternalOutput").ap()

    with tile.TileContext(nc) as tc:
        _emit(tc, xT, wpT, bvr, out)
    nc.compile()
    return nc


_NC = None


def _get_nc():
    global _NC
    if _NC is None:
        _NC = build()
    return _NC


def make_in_maps(x, Wq, bq, Wk, bk, Wv, bv):
    # bq cancels under the softmax-over-queries (see module docstring)
    Wq64 = np.asarray(Wq, np.float64)
    Wk64 = np.asarray(Wk, np.float64)
    P = (Wk64.T @ Wq64).astype(np.float32)          # [d2, d]
    g = (np.asarray(bk, np.float64) @ Wq64).astype(np.float32).reshape(D, 1)
    # pack to the on-core SBUF layout [128, 1026]:
    #   [P_t0 | P_t1 | g_t0 | g_t1 | wv_t0 | wv_t1]  (t = d-row half)
    # so the weight DMA is contiguous runs and the P/g part loads first
    WvT = np.asarray(Wv, np.float32).T * VSCALE
    wpT = np.empty((128, 1026), np.float32)
    wpT[:, 0:256] = P[0:128]
    wpT[:, 256:512] = P[128:256]
    wpT[:, 512] = g[0:128, 0]
    wpT[:, 513] = g[128:256, 0]
    wpT[:, 514:770] = WvT[0:128]
    wpT[:, 770:1026] = WvT[128:256]
    wpT = np.ascontiguousarray(wpT).astype(BF16)
    bvr = (np.asarray(bv, np.float32) * VSCALE).reshape(1, D).astype(BF16)
    in_maps = []
    for core in range(NCORES):
        b, h = divmod(core, 2)
        xTb = np.asarray(x[b]).T.astype(BF16)
        if h:  # rotate so this core's keys are always columns 0:KH
            xTb = np.concatenate([xTb[:, KH:], xTb[:, :KH]], axis=1)
        in_maps.append({
            "xT": np.ascontiguousarray(xTb),
            "wpT": wpT, "bvr": bvr,
        })
    return in_maps


def run(x, Wq, bq, Wk, bk, Wv, bv, trace=False):
    """Run on the 8 cores; returns (full_output, BassKernelResults)."""
    nc = _get_nc()
    in_maps = make_in_maps(x, Wq, bq, Wk, bk, Wv, bv)
    res = run_bass_kernel_spmd(nc, in_maps, core_ids=list(range(NCORES)),
                               trace=trace)
    outs = []
    for b in range(B):
        p0 = res.results[2 * b]["out"].astype(np.float32)      # [D, S]
        p1 = res.results[2 * b + 1]["out"].astype(np.float32)  # [D, S] rotated
        p1 = np.concatenate([p1[:, KH:], p1[:, :KH]], axis=1)  # undo rotation
        outs.append((p0 + p1).T * (1.0 / VSCALE))
    return np.stack(outs).astype(np.float32), res


def kernel(x, Wq, bq, Wk, bk, Wv, bv):
    full, _ = run(x, Wq, bq, Wk, bk, Wv, bv, trace=False)
    return full
